# revision 13
# baseline (speedup 1.0000x reference)
"""MoE dispatcher kernel for Trainium2 (8 NeuronCores, expert-parallel).

Contract: kernel(**inputs) takes FULL inputs and returns the FULL output.

Strategy (expert-parallel, one expert per core):
  - host: softmax(gate_logits) -> top-2 -> combine weights per (token, expert)
  - host "all-to-all dispatch": for expert e, gather its routed tokens,
    pre-scale rows by the combine weight (w * (x @ W) == (w*x) @ W), pad to a
    common capacity C, transpose to [D, C] so the device streams tokens along
    the free dim.  One expert per core.
  - device (per core): Y^T[D,C] = W[e]^T @ X^T via PE array.  Loop nest is
    (m-tile-pair, k, m-in-pair, n-tile); X arrives in k-major chunks split
    across BOTH HWDGE rings (sync+scalar) interleaved with the pair-0 W
    chunks, so the stream starts at ~7us and is never input-starved; later
    pair W chunks follow on the sync ring in use order.
  - PSUM evicted per (pair, m) into a [P, C] bf16 staging tile; output DMAs
    alternate rings so the final pair drains in parallel (bf16 output halves
    write traffic; rel-err budget is 2e-2).
  - host "all-to-all combine": scatter-add each expert's Y rows back to the
    token axis (plain add; weights were folded into x).

DRAM layouts (host-permuted so every DMA is contiguous per partition):
  x   [P, KT*C]        x[p, k*C + n]                 = X^T[k*128 + p, n]
  w   [P, MT/2*KT*256] w[p, ((pr*KT)+k)*256 + j2*128 + j]
                        = W[e][k*128 + p, (pr*2 + j2)*128 + j]
  yt  [MT, P, C]       yt[mi, p, n]                  = Y^T[mi*128 + p, n]
"""

import os

import numpy as np

N_CORES = 8
P = 128
NSPLIT = 512  # PSUM bank / max fp32 moving free dim
WARM_LDW = int(os.environ.get("BASS_MOE_WARM_LDW", "24"))

_prog_cache: dict = {}


def _np_bf16():
    import ml_dtypes

    return ml_dtypes.bfloat16


def _n_tiles(C):
    """Split C into column tiles of at most NSPLIT (one PSUM bank each)."""
    out = []
    n0 = 0
    while n0 < C:
        sz = min(NSPLIT, C - n0)
        out.append((n0, sz))
        n0 += sz
    return out


def _build_program(D: int, C: int):
    import concourse.bacc as bacc
    import concourse.mybir as mybir
    import concourse.tile as tile

    bf16 = mybir.dt.bfloat16
    f32 = mybir.dt.float32
    KT = D // P  # contraction tiles
    MT = D // P  # output-feature tiles
    NPR = MT // 2  # m-tile pairs
    n_tiles = _n_tiles(C)
    NT = len(n_tiles)
    assert 2 * NT <= 8, "PSUM banks: need 2*NT <= 8"

    nc = bacc.Bacc(None, target_bir_lowering=False)
    x = nc.declare_dram_parameter("x", [P, KT * C], bf16, isOutput=False)
    w = nc.declare_dram_parameter("w", [P, NPR * KT * 256], bf16, isOutput=False)
    yt = nc.declare_dram_parameter("yt", [MT, P, C], bf16, isOutput=True)

    with tile.TileContext(nc) as tc:
        with (
            tc.tile_pool(name="xpool", bufs=1) as xpool,
            tc.tile_pool(name="wpool", bufs=NPR) as wpool,
            tc.tile_pool(name="psum", bufs=8, space="PSUM") as psum_pool,
            tc.tile_pool(name="stage", bufs=4) as stpool,
            tc.tile_pool(name="warm", bufs=1) as warmpool,
        ):
            # Pre-warm the PE's HAM clock gate with standalone LDWEIGHTS while
            # the first input chunks stream in (warm matmuls get dropped by
            # the backend as dead stores; bare LDWEIGHTS survive and count as
            # PE activity).  They're overwritten by the first real LDWEIGHTS.
            wt = warmpool.tile([P, P], bf16, tag="warm_w")
            nc.gpsimd.memset(wt[:], 0.0)
            for _ in range(WARM_LDW):
                nc.tensor.ldweights(wt[:])

            # --- input DMAs, issued in consumption order across BOTH rings
            # (they drain concurrently, packet round-robin on the 16 SDMA
            # engines).  The first chunks are small: the first matmul trio
            # needs only x[k0,t0] + W[pr0,k0], so a latency-ladder gets the
            # stream started ~2us earlier than full-size chunks would.
            w_sb = []
            for pr in range(NPR):
                twl = wpool.tile([P, KT, 256], bf16, tag="w_sb", name="w_sb")
                w_sb.append(twl)

            def w_dma(pr, k0, k1):
                nc.sync.dma_start(
                    w_sb[pr][:, k0:k1, :].rearrange("p k j -> p (k j)"),
                    w[:, (pr * KT + k0) * 256 : (pr * KT + k1) * 256],
                )

            x_sb = xpool.tile([P, KT, C], bf16, tag="x_sb")

            def x_dma(eng, k, n0=0, n1=C):
                eng.dma_start(
                    x_sb[:, k, n0:n1],
                    x[:, k * C + n0 : k * C + n1],
                )

            NT0 = n_tiles[0][1]
            x_dma(nc.scalar, 0, 0, NT0)  # x[k0, t0]
            w_dma(0, 0, 1)  # W[pr0, k0]
            x_dma(nc.scalar, 0, NT0, C)  # x[k0, rest]
            w_dma(0, 1, 2)  # W[pr0, k1]
            x_dma(nc.scalar, 2)
            x_dma(nc.sync, 1)
            x_dma(nc.scalar, 4)
            w_dma(0, 2, KT)  # W[pr0, k2-7]
            x_dma(nc.scalar, 6)
            x_dma(nc.sync, 3)
            x_dma(nc.sync, 5)
            x_dma(nc.sync, 7)
            for pr in range(1, NPR):
                w_dma(pr, 0, KT)

            for pr in range(NPR):
                ps = [
                    [
                        psum_pool.tile([P, NSPLIT], f32, tag="ps", name="ps")
                        for _ in n_tiles
                    ]
                    for _ in range(2)
                ]
                for k in range(KT):
                    for j2 in range(2):
                        lhsT = w_sb[pr][:, k, j2 * P : (j2 + 1) * P]
                        for t, (n0, nsz) in enumerate(n_tiles):
                            nc.tensor.matmul(
                                ps[j2][t][:, :nsz],
                                lhsT=lhsT,
                                rhs=x_sb[:, k, n0 : n0 + nsz],
                                start=(k == 0),
                                stop=(k == KT - 1),
                            )
                for j2 in range(2):
                    stage = stpool.tile([P, C], bf16, tag="stage", name="stage")
                    for t, (n0, nsz) in enumerate(n_tiles):
                        nc.vector.tensor_copy(
                            stage[:, n0 : n0 + nsz], ps[j2][t][:, :nsz]
                        )
                    # Alternate output rings so the final pair's two output
                    # DMAs drain in parallel instead of serializing.
                    eng = nc.scalar if j2 == 0 else nc.sync
                    eng.dma_start(yt[pr * 2 + j2, :, :], stage[:])
    nc.compile()
    return nc


def kernel(hidden: np.ndarray, gate_logits: np.ndarray, W: np.ndarray) -> np.ndarray:
    from concourse.bass_utils import run_bass_kernel_spmd

    hidden = np.asarray(hidden)
    gate_logits = np.asarray(gate_logits)
    W = np.asarray(W)
    B, S, D = hidden.shape
    T, E = gate_logits.shape
    assert E == N_CORES
    KT = D // P
    MT = D // P
    NPR = MT // 2
    bf16 = _np_bf16()
    x = np.ascontiguousarray(hidden.reshape(T, D).astype(np.float32))

    # --- routing on host (fp32, matches reference softmax/top-2) ---
    g = gate_logits.astype(np.float32)
    m = g.max(axis=-1, keepdims=True)
    p = np.exp(g - m)
    p /= p.sum(axis=-1, keepdims=True)
    top2 = np.argpartition(-p, 1, axis=-1)[:, :2]

    routed = [np.nonzero((top2 == e).any(axis=1))[0] for e in range(E)]
    counts = np.array([len(r) for r in routed])
    C = max(NSPLIT, int(-(-counts.max() // P)) * P)  # capacity, multiple of 128

    in_maps = []
    for e in range(E):
        idx = routed[e]
        cnt = len(idx)
        scale = p[idx, e].astype(np.float32)
        xe = (x[idx] * scale[:, None]).astype(bf16)  # [cnt, D]
        # x dram [P, KT*C]: x[p, k*C+n] = Xe^T[k*128+p, n]
        xk = np.zeros((KT, P, C), dtype=bf16)
        xk[:, :, :cnt] = xe.T.reshape(KT, P, cnt)
        x_dram = np.ascontiguousarray(xk.transpose(1, 0, 2).reshape(P, KT * C))
        # w dram [P, NPR*KT*256]: w[p, (pr*KT+k)*256 + j2*128 + j]
        #   = W[e][k*128+p, (pr*2+j2)*128+j]
        Wb = W[e].astype(bf16).reshape(KT, P, MT, P)
        w_dram = np.ascontiguousarray(
            Wb.transpose(1, 2, 0, 3)  # [p, mi, k, j]
            .reshape(P, NPR, 2, KT, P)
            .transpose(0, 1, 3, 2, 4)  # [p, pr, k, j2, j]
            .reshape(P, NPR * KT * 256)
        )
        in_maps.append({"x": x_dram, "w": w_dram})

    key = (D, C)
    if key not in _prog_cache:
        _prog_cache[key] = _build_program(D, C)
    nc = _prog_cache[key]

    res = run_bass_kernel_spmd(nc, in_maps, core_ids=list(range(N_CORES)))

    # --- combine on host ---
    out = np.zeros((T, D), dtype=np.float32)
    for e in range(E):
        idx = routed[e]
        cnt = len(idx)
        ye_t = np.asarray(res.results[e]["yt"]).reshape(D, C)  # Y^T
        out[idx] += ye_t[:, :cnt].T.astype(np.float32)
    return out.reshape(B, S, D)
